# revision 53
# baseline (speedup 1.0000x reference)
"""Trainium2 Bass kernel for nn_BaseGenerator (4-layer dense transformer).

Strategy: pure data-parallel over batch (B=8 -> 8 NeuronCores, no
collectives).  Each core runs the full transformer on one batch element.
Activations are kept feature-major [E, S] in bf16 so every GEMM contracts
over the partition dim; PSUM accumulates in fp32.

Optimizations vs the original baseline (1130us -> ~1005us):
  - scores: K=64 matmuls for head pairs run concurrently on PE row groups
    (0,0)/(64,0); causal column trim (q >= kc*128) on scores/mask/exp/AV;
    kc2+kc3 share one psum bank so each head needs only 3 exp ops.
  - mask packed to [H, 128, 1280] (valid causal region only), preloaded
    into psum (start=True) so score matmuls close each accumulation group.
  - out-proj: two heads' ctx packed into one [128, S] tile -> K=128 mms;
    attention gets the full 8-bank psum budget (scores ride the gemm pool).
  - V bias folded into bo on host (bo' = bo + Wo @ bv).
  - LN: sums via M=128 all-ones matmuls (M=1 psum-row accumulation has a
    ~5x HW serialization penalty), rstd = exp(-0.5*ln(v)) with the
    activation-table patch pinning Exp/Ln to the natural_log_exp set
    (avoids ~2.7us table swaps on the LN critical path), broadcasts via
    K=1 matmuls, sum matmuls overlapped into out-proj B / FFN2 half 1.
  - embedding: one K=94 matmul per chunk (val/ring one-hots stacked).
  - weight/mask DMA prefetching across phase boundaries; bf16 logits.
"""

import os
import sys

for _p in ("/opt/trn_rl_repo",):
    if _p not in sys.path:
        sys.path.insert(0, _p)

import ml_dtypes
import numpy as np

import concourse.bass as bass
import concourse.mybir as mybir
import concourse.tile as tile
from concourse import bacc
from concourse.bass_utils import run_bass_kernel_spmd

BF16 = ml_dtypes.bfloat16

L, E, H, F = 4, 1024, 16, 4096
B, S = 8, 512
VV, VR = 40, 30
DIST_V = 200
PAD_ID = 0
DH = E // H  # 64
NE = E // 128  # 8 feature chunks
NO = 10  # logit row tiles (1280 padded)
NEG = -1.0e9

# causal column layout: for k-chunk kc, valid q range is [kc*128, 512)
KOFF = [0, 512, 896, 1152]
KW = [512, 384, 256, 128]
MASKW = 1280

f32 = mybir.dt.float32
bf16 = mybir.dt.bfloat16
AF = mybir.ActivationFunctionType
OP = mybir.AluOpType

_CACHE = {}


# ----------------------------------------------------------------------------
# host-side input prep
# ----------------------------------------------------------------------------

def _prep_shared(inp):
    """Weight-layout prep shared by all cores. Returns dict name->np array."""
    out = {}

    def b16(x):
        return np.ascontiguousarray(x.astype(BF16))

    Wqkv = np.asarray(inp["Wqkv"], np.float32).copy()  # [L, 3E, E]
    bqkv = np.asarray(inp["bqkv"], np.float32).copy()  # [L, 3E]
    bv = bqkv[:, 2 * E:].copy()  # [L, E] (V bias, folded into bo below)
    # fold attention scale into Q projection
    scale = 1.0 / np.sqrt(DH)
    Wqkv[:, :E, :] *= scale
    bqkv[:, :E] *= scale

    def block_lhsT(W, gsize):
        # W: [L?, OUT, IN] -> [.., G, 128, IN//128, gsize] with
        # out[..., g, p, c, o] = W[..., g*gsize + o, c*128 + p]
        *lead, O, I = W.shape
        G = O // gsize
        nc_ = I // 128
        Wb = W.reshape(*lead, G, gsize, nc_, 128)
        Wb = np.moveaxis(Wb, -1, -3)  # [..., G, 128, gsize, nc]
        Wb = np.swapaxes(Wb, -1, -2)  # [..., G, 128, nc, gsize]
        return np.ascontiguousarray(Wb)

    out["wqkv"] = b16(block_lhsT(Wqkv, 512))          # [L, 6, 128, 8, 512]

    # Wo head-pair major: wo2[l, p, hp, mt, o] = Wo[l, mt*128+o, hp*128+p]
    Wo = np.asarray(inp["Wo"], np.float32)  # [L, E(out), E(in=ctx)]
    t = Wo.reshape(L, 8, 128, 8, 128)       # [l, mt, o, hp, p]
    out["wo2"] = b16(t.transpose(0, 4, 3, 1, 2))  # [l, p, hp, mt, o]

    # bo' = bo + Wo @ bv  (ctx rows are normalized, sum of attn weights = 1)
    bo2 = np.asarray(inp["bo"], np.float32) + np.einsum("loi,li->lo", Wo, bv)

    out["w1"] = b16(block_lhsT(np.asarray(inp["W1"], np.float32), 512))  # [L,8,128,8,512]
    W2 = np.asarray(inp["W2"], np.float32)  # out=E, in=F
    w2b = block_lhsT(W2, 512)  # [L, 2, 128, 32, 512]
    w2b = w2b.reshape(L, 2, 128, 4, 8, 512).transpose(0, 1, 3, 2, 4, 5)
    out["w2"] = b16(w2b)  # [L, 2, 4, 128, 8, 512]

    genW = np.asarray(inp["gen_W"], np.float32)  # [1200, E]
    genW_pad = np.zeros((1280, E), np.float32)
    genW_pad[:1200] = genW
    out["genw"] = b16(block_lhsT(genW_pad, 640))  # [2, 128, 8, 640]

    gen_b = np.asarray(inp["gen_b"], np.float32)
    gbp = np.zeros((1280,), np.float32)
    gbp[:1200] = gen_b
    out["gen_b_pp"] = np.ascontiguousarray(gbp.reshape(NO, 128).T)  # [128, 10]

    def pp(v):  # [..., N*128] -> [..., 128, N]
        *lead, N = v.shape
        return np.ascontiguousarray(
            v.reshape(*lead, N // 128, 128).swapaxes(-1, -2).astype(np.float32)
        )

    out["bqkv_pp"] = pp(bqkv[:, : 2 * E])  # [L, 128, 16] (Q scaled)
    out["bo_pp"] = pp(bo2)  # [L, 128, 8]
    out["b1_pp"] = pp(np.asarray(inp["b1"], np.float32))  # [L, 128, 32]
    out["b2_pp"] = pp(np.asarray(inp["b2"], np.float32))  # [L, 128, 8]

    ln_s = np.stack([np.asarray(inp["ln1_s"], np.float32),
                     np.asarray(inp["ln2_s"], np.float32)], 1)  # [L, 2, E]
    ln_b = np.stack([np.asarray(inp["ln1_b"], np.float32),
                     np.asarray(inp["ln2_b"], np.float32)], 1)
    out["ln_s_pp"] = pp(ln_s)  # [L, 2, 128, 8]
    out["ln_b_pp"] = pp(ln_b)
    out["lnf_s_pp"] = pp(np.asarray(inp["lnf_s"], np.float32))  # [128, 8]
    out["lnf_b_pp"] = pp(np.asarray(inp["lnf_b"], np.float32))

    # stacked embedding table: rows 0:40 val, 40:64 zero, 64:94 ring
    embcat = np.zeros((94, E), np.float32)
    embcat[0:VV] = np.asarray(inp["val_emb"], np.float32)
    embcat[64:64 + VR] = np.asarray(inp["ring_emb"], np.float32)
    out["embcat"] = b16(embcat)

    # iota for the stacked one-hot: 0..39 | -1 x24 | 0..29 | -1 x34
    iota94 = np.full((128, 1), -1.0, np.float32)
    iota94[0:VV, 0] = np.arange(VV)
    iota94[64:64 + VR, 0] = np.arange(VR)
    out["iota94"] = np.ascontiguousarray(iota94)

    out["id128"] = b16(np.eye(128, dtype=np.float32))
    out["ones_col"] = b16(np.ones((128, 1), np.float32))
    return out


def _prep_percore(inp):
    """Per-core tensors: token rows + packed causal attention mask."""
    val = np.asarray(inp["val_sequences"]).astype(np.int64)    # [B, S]
    ring = np.asarray(inp["ring_sequences"]).astype(np.int64)  # [B, S]
    dist = np.asarray(inp["distance_squares"]).astype(np.int64)  # [B, S, S]
    de = np.asarray(inp["dist_emb"], np.float32)  # [200, H]

    # mask[b, h, k, q] = de[dist[b, q, k], h] or NEG
    m = de[dist]                         # [B, S(q), S(k), H]
    m = m.transpose(0, 3, 2, 1)          # [B, H, k, q]
    kk = np.arange(S)
    causal = kk[:, None] <= kk[None, :]  # [k, q] keep where k <= q
    m = np.where(causal[None, None], m, NEG)
    padk = val == PAD_ID  # [B, S]
    m = np.where(padk[:, None, :, None], NEG, m)
    # pack causal region: [B, H, 128, 1280]; chunk kc covers q in [kc*128,512)
    mp = np.empty((B, H, 128, MASKW), np.float32)
    for kc in range(4):
        mp[:, :, :, KOFF[kc]:KOFF[kc] + KW[kc]] = (
            m[:, :, kc * 128:(kc + 1) * 128, kc * 128:]
        )
    mp = np.ascontiguousarray(mp.astype(BF16))

    cores = []
    for b in range(B):
        cores.append({
            "mask": mp[b],
            "valrow": np.ascontiguousarray(val[b].reshape(1, S).astype(BF16)),
            "ringrow": np.ascontiguousarray(ring[b].reshape(1, S).astype(BF16)),
        })
    return cores


# ----------------------------------------------------------------------------
# device program
# ----------------------------------------------------------------------------

def _declare(nc):
    d = {}

    def di(name, shape, dt):
        d[name] = nc.dram_tensor(name, list(shape), dt, kind="ExternalInput").ap()

    di("wqkv", (L, 6, 128, 8, 512), bf16)
    di("wo2", (L, 128, 8, 8, 128), bf16)
    di("w1", (L, 8, 128, 8, 512), bf16)
    di("w2", (L, 2, 4, 128, 8, 512), bf16)
    di("genw", (2, 128, 8, 640), bf16)
    di("gen_b_pp", (128, NO), f32)
    di("bqkv_pp", (L, 128, 16), f32)
    di("bo_pp", (L, 128, 8), f32)
    di("b1_pp", (L, 128, 32), f32)
    di("b2_pp", (L, 128, 8), f32)
    di("ln_s_pp", (L, 2, 128, 8), f32)
    di("ln_b_pp", (L, 2, 128, 8), f32)
    di("lnf_s_pp", (128, 8), f32)
    di("lnf_b_pp", (128, 8), f32)
    di("embcat", (94, E), bf16)
    di("iota94", (128, 1), f32)
    di("id128", (128, 128), bf16)
    di("ones_col", (128, 1), bf16)
    di("mask", (H, 128, MASKW), bf16)
    di("valrow", (1, S), bf16)
    di("ringrow", (1, S), bf16)
    d["logits"] = nc.dram_tensor(
        "logits", [NO, 128, S], bf16, kind="ExternalOutput"
    ).ap()
    if os.environ.get("BG_DEBUG"):
        def do(name, shape):
            d[name] = nc.dram_tensor(name, list(shape), bf16,
                                     kind="ExternalOutput").ap()
        do("dbg_h0", (NE, 128, S))
        do("dbg_qk", (16, 128, S))
        do("dbg_v", (4, 128, H, DH + 1))
        do("dbg_at", (2, 128, MASKW))
        do("dbg_ctx", (8, 128, S))
        do("dbg_r1", (NE, 128, S))
        do("dbg_h1", (NE, 128, S))
        do("dbg_h2", (NE, 128, S))
    return d


def _emit(nc, tc, d, ctx):
    mm = nc.tensor.matmul

    cpool = ctx.enter_context(tc.tile_pool(name="cpool", bufs=1))
    wpool = ctx.enter_context(tc.tile_pool(name="wpool", bufs=4))
    wopool = ctx.enter_context(tc.tile_pool(name="wopool", bufs=1))
    hpool = ctx.enter_context(tc.tile_pool(name="hpool", bufs=17))
    qkpool = ctx.enter_context(tc.tile_pool(name="qkpool", bufs=16))
    vpool = ctx.enter_context(tc.tile_pool(name="vpool", bufs=5))
    maskpool = ctx.enter_context(tc.tile_pool(name="maskpool", bufs=4))
    atpool = ctx.enter_context(tc.tile_pool(name="atpool", bufs=4))
    ctxpool = ctx.enter_context(tc.tile_pool(name="ctxpool", bufs=9))
    ffpool = ctx.enter_context(tc.tile_pool(name="ffpool", bufs=33))
    tmppool = ctx.enter_context(tc.tile_pool(name="tmppool", bufs=4))
    smallf = ctx.enter_context(tc.tile_pool(name="smallf", bufs=5))
    smallb = ctx.enter_context(tc.tile_pool(name="smallb", bufs=4))
    recpool = ctx.enter_context(tc.tile_pool(name="recpool", bufs=3))
    lnbpool = ctx.enter_context(tc.tile_pool(name="lnbpool", bufs=4))
    outpool = ctx.enter_context(tc.tile_pool(name="outpool", bufs=2))
    pppool = ctx.enter_context(tc.tile_pool(name="pppool", bufs=4))

    ps_gemm = ctx.enter_context(tc.tile_pool(name="ps_gemm", bufs=4, space="PSUM"))
    ps_score = ctx.enter_context(tc.tile_pool(name="ps_score", bufs=2, space="PSUM"))
    ps_ctx = ctx.enter_context(tc.tile_pool(name="ps_ctx", bufs=2, space="PSUM"))

    hw = nc.sync  # HWDGE dma engine

    # --- constants -----------------------------------------------------------
    id128 = cpool.tile([128, 128], bf16)
    hw.dma_start(out=id128, in_=d["id128"])
    ones_col = cpool.tile([128, 1], bf16)
    hw.dma_start(out=ones_col, in_=d["ones_col"])
    iota94 = cpool.tile([128, 1], f32)
    hw.dma_start(out=iota94, in_=d["iota94"])
    embcat = cpool.tile([94, E], bf16)
    hw.dma_start(out=embcat, in_=d["embcat"])
    genb_pp = cpool.tile([128, NO], f32)
    hw.dma_start(out=genb_pp, in_=d["gen_b_pp"])
    lnf_s = cpool.tile([128, 8], f32)
    hw.dma_start(out=lnf_s, in_=d["lnf_s_pp"])
    lnf_b = cpool.tile([128, 8], f32)
    hw.dma_start(out=lnf_b, in_=d["lnf_b_pp"])
    eps_t = cpool.tile([128, 1], f32)
    nc.vector.memset(eps_t, 1e-5)
    ones_r1 = cpool.tile([1, 128], bf16)
    nc.vector.memset(ones_r1, 1.0)
    ones128 = cpool.tile([128, 128], bf16)
    nc.vector.memset(ones128, 1.0)
    # prefetch first QKV weight group of layer 0 (hides DMA cold-start)
    wq_pre = wpool.tile([128, 8, 512], bf16, tag="w")
    hw.dma_start(out=wq_pre, in_=d["wqkv"][0, 0])

    # --- embedding -----------------------------------------------------------
    with nc.named_scope("embed"):
        vr = tmppool.tile([94, S], bf16, tag="sq")
        nc.vector.memset(vr[32:64, :], -2.0)
        nc.gpsimd.dma_start(out=vr[0:VV, :], in_=d["valrow"].to_broadcast((VV, S)))
        nc.gpsimd.dma_start(out=vr[64:64 + VR, :],
                            in_=d["ringrow"].to_broadcast((VR, S)))
        oh = tmppool.tile([94, S], bf16, tag="tmp")
        nc.vector.tensor_scalar(oh, vr, iota94[0:94, :], None, OP.is_equal)

        h_t = []
        for c in range(NE):
            ps = ps_gemm.tile([128, S], f32, tag="gemm")
            mm(ps, embcat[:, c * 128:(c + 1) * 128], oh, start=True, stop=True)
            ht = hpool.tile([128, S], bf16, tag="h")
            nc.scalar.activation(ht, ps, AF.Copy, scale=float(np.sqrt(E)))
            if "dbg_h0" in d:
                hw.dma_start(out=d["dbg_h0"][c], in_=ht)
            h_t.append(ht)

    # --- layers --------------------------------------------------------------
    env = dict(locals())
    for l in range(L):
        h_t, wq_pre = _layer(nc, tc, d, l, h_t, env, wq_pre)

    # --- final LN + head -----------------------------------------------------
    with nc.named_scope("final"):
        genw_sb = []
        for g in range(2):
            wt = wpool.tile([128, 8, 640], bf16, tag="w")
            hw.dma_start(out=wt, in_=d["genw"][g])
            genw_sb.append(wt)
        hf = _layernorm(nc, d, h_t, lnf_s, lnf_b, env, "lnf")
        for mt in range(NO):
            g, mi = divmod(mt, 5)
            ps = ps_gemm.tile([128, S], f32, tag="gemm")
            for c in range(NE):
                mm(ps, genw_sb[g][:, c, mi * 128:(mi + 1) * 128], hf[c],
                   start=(c == 0), stop=(c == NE - 1))
            ot = outpool.tile([128, S], bf16, tag="bfout")
            nc.scalar.activation(ot, ps, AF.Identity, bias=genb_pp[:, mt:mt + 1])
            hw.dma_start(out=d["logits"][mt], in_=ot)


def _layernorm(nc, d, r_t, s_pp, b_pp, env, nm):
    """r_t: 8 bf16 [128, S] feature-major tiles -> returns normalized tiles.

    sums via 2-way col-tiled M=1 matmuls; rstd = exp(-0.5 * ln(var + eps))
    to stay inside the natural_log_exp activation table set.
    """
    st = _ln_begin(env, nm)
    _ln_sums(nc, env, st, r_t, 0, NE)
    return _ln_finish(nc, d, st, r_t, s_pp, b_pp, env)


def _ln_begin(env, nm):
    """Allocate the two psum stat accumulators (no instructions)."""
    ps_ctx = env["ps_ctx"]
    psS = ps_ctx.tile([128, S], f32, tag="ctxps", name=f"{nm}_psS")
    psQ = ps_ctx.tile([128, S], f32, tag="ctxps", name=f"{nm}_psQ")
    return {"psS": psS, "psQ": psQ, "nm": nm}


def _ln_sums(nc, env, st, r_t, c0, c1):
    """Emit sum/sumsq matmuls for chunks [c0, c1). M=128 all-ones lhsT:
    one matmul per chunk at full rate (M=1 accumulation into a single
    psum row has a ~5x serialization penalty on HW); every psum partition
    holds the full sum."""
    mm = nc.tensor.matmul
    tmppool = env["tmppool"]
    for c in range(c0, c1):
        sq = tmppool.tile([128, S], bf16, tag="sq", name=f"{st['nm']}_sq{c}")
        nc.vector.tensor_mul(sq, r_t[c], r_t[c])
        mm(st["psS"], env["ones128"], r_t[c],
           start=(c == 0), stop=(c == NE - 1))
        mm(st["psQ"], env["ones128"], sq,
           start=(c == 0), stop=(c == NE - 1))


def _ln_finish(nc, d, st, r_t, s_pp, b_pp, env):
    mm = nc.tensor.matmul
    ps_score = env["ps_score"]
    smallf = env["smallf"]; smallb = env["smallb"]
    tmppool = env["tmppool"]; hpool = env["hpool"]; lnbpool = env["lnbpool"]
    psS, psQ, nm = st["psS"], st["psQ"], st["nm"]

    s2 = smallf.tile([1, S], f32, tag="sf", name=f"{nm}_s2")
    nc.scalar.activation(s2, psS[0:1, :], AF.Square)
    varE = smallf.tile([1, S], f32, tag="sf", name=f"{nm}_varE")
    # varE = sumsq - s2/E  (= E * var)
    nc.vector.scalar_tensor_tensor(varE, s2, -1.0 / E, psQ[0:1, :],
                                   OP.mult, OP.add)
    # HAM warmth tickle mid-chain (depends on varE via a tiny bf16 copy)
    vb = smallb.tile([1, 4], bf16, tag="vb")
    nc.vector.tensor_copy(vb, varE[:, 0:4])
    tick = ps_score.tile([128, 4], f32, tag="score", name=f"{nm}_t0")
    mm(tick, env["ones_r1"], vb, start=True, stop=True)

    lnv = smallf.tile([1, S], f32, tag="sf", name=f"{nm}_lnv")
    nc.scalar.activation(lnv, varE, AF.Ln, scale=1.0 / E,
                         bias=env["eps_t"][0:1, :])
    rstd = smallf.tile([1, S], f32, tag="sf", name=f"{nm}_rstd")
    nc.scalar.activation(rstd, lnv, AF.Exp, scale=-0.5)

    rstd_b = smallb.tile([1, S], bf16, tag="sb")
    nc.vector.tensor_copy(rstd_b, rstd)
    u_b = smallb.tile([1, S], bf16, tag="sb")
    # u = mean * rstd = (sum/E) * rstd
    nc.vector.scalar_tensor_tensor(u_b, psS[0:1, :], 1.0 / E, rstd,
                                   OP.mult, OP.mult)

    # broadcast rstd / u to 128 partitions with K=1 matmuls, then copy to
    # SBUF bf16 so the tail DVE ops run at 2x 16-bit rate
    rstdR_ps = ps_score.tile([128, S], f32, tag="score", name=f"{nm}_rstdRp")
    mm(rstdR_ps, env["ones_r1"], rstd_b, start=True, stop=True)
    uR_ps = ps_score.tile([128, S], f32, tag="score", name=f"{nm}_uRp")
    mm(uR_ps, env["ones_r1"], u_b, start=True, stop=True)
    rstdR = lnbpool.tile([128, S], bf16, tag="lnb", name=f"{nm}_rstdR")
    nc.scalar.activation(rstdR, rstdR_ps, AF.Copy)
    uR = lnbpool.tile([128, S], bf16, tag="lnb", name=f"{nm}_uR")
    nc.vector.tensor_copy(uR, uR_ps)

    out_t = []
    for c in range(NE):
        t1 = tmppool.tile([128, S], bf16, tag="tmp")
        nc.vector.tensor_mul(t1, r_t[c], rstdR)
        t2 = tmppool.tile([128, S], bf16, tag="sq")
        nc.vector.tensor_sub(t2, t1, uR)
        ht = hpool.tile([128, S], bf16, tag="h")
        nc.scalar.activation(ht, t2, AF.Identity,
                             bias=b_pp[:, c:c + 1], scale=s_pp[:, c:c + 1])
        out_t.append(ht)
    return out_t


def _layer(nc, tc, d, l, h_t, env, wq0):
    mm = nc.tensor.matmul
    hw = nc.sync
    wpool = env["wpool"]; wopool = env["wopool"]; hpool = env["hpool"]
    qkpool = env["qkpool"]; vpool = env["vpool"]; maskpool = env["maskpool"]
    atpool = env["atpool"]; ctxpool = env["ctxpool"]; ffpool = env["ffpool"]
    tmppool = env["tmppool"]; smallf = env["smallf"]; recpool = env["recpool"]
    pppool = env["pppool"]
    ps_gemm = env["ps_gemm"]; ps_score = env["ps_score"]; ps_ctx = env["ps_ctx"]
    id128 = env["id128"]; ones_col = env["ones_col"]

    # per-layer small params
    bqkv_pp = pppool.tile([128, 16], f32, tag="pp16")
    hw.dma_start(out=bqkv_pp, in_=d["bqkv_pp"][l])
    bo_pp = pppool.tile([128, 8], f32, tag="pp8")
    hw.dma_start(out=bo_pp, in_=d["bo_pp"][l])
    b1_pp = pppool.tile([128, 32], f32, tag="pp32")
    hw.dma_start(out=b1_pp, in_=d["b1_pp"][l])
    b2_pp = pppool.tile([128, 8], f32, tag="pp8")
    hw.dma_start(out=b2_pp, in_=d["b2_pp"][l])
    ln_s = [pppool.tile([128, 8], f32, tag="pp8", name=f"lns{l}_{i}")
            for i in range(2)]
    ln_b = [pppool.tile([128, 8], f32, tag="pp8", name=f"lnb{l}_{i}")
            for i in range(2)]
    for i in range(2):
        hw.dma_start(out=ln_s[i], in_=d["ln_s_pp"][l, i])
        hw.dma_start(out=ln_b[i], in_=d["ln_b_pp"][l, i])
    mask_sb = {}

    def load_masks(j):
        for h in (2 * j, 2 * j + 1):
            mk = maskpool.tile([128, MASKW], bf16, tag="mask",
                               name=f"mk{l}_{h}")
            hw.dma_start(out=mk, in_=d["mask"][h])
            mask_sb[h] = mk

    # --- QKV -----------------------------------------------------------------
    with nc.named_scope(f"L{l}_qkv"):
        qk_t = []  # 16 tiles: q 0..7, k 8..15
        for g in range(4):  # Q, K feature-major
            if g == 0:
                wt = wq0  # prefetched during previous layer's FFN
            else:
                wt = wpool.tile([128, 8, 512], bf16, tag="w")
                hw.dma_start(out=wt, in_=d["wqkv"][l, g])
            if g == 1:
                # out-proj weights: DMA behind the early QKV weights so the
                # first QKV matmuls are never starved
                wo_sb = wopool.tile([128, 8, 8, 128], bf16, tag="wo")
                hw.dma_start(out=wo_sb, in_=d["wo2"][l])
            for mi in range(4):
                mt = g * 4 + mi
                ps = ps_gemm.tile([128, S], f32, tag="gemm")
                for c in range(NE):
                    mm(ps, wt[:, c, mi * 128:(mi + 1) * 128], h_t[c],
                       start=(c == 0), stop=(c == NE - 1))
                qk = qkpool.tile([128, S], bf16, tag="qk")
                nc.scalar.activation(qk, ps, AF.Identity,
                                     bias=bqkv_pp[:, mt:mt + 1])
                if l == 0 and "dbg_qk" in d:
                    hw.dma_start(out=d["dbg_qk"][mt], in_=qk)
                qk_t.append(qk)
        # V token-major [128, H, DH+1] with ones column (softmax denominator)
        v_t = []
        for n in range(4):
            vt = vpool.tile([128, H, DH + 1], bf16, tag="v", name=f"v{l}_{n}")
            nc.vector.memset(vt[:, :, DH:DH + 1], 1.0)
            v_t.append(vt)
        for g in range(2):
            wt = wpool.tile([128, 8, 512], bf16, tag="w")
            hw.dma_start(out=wt, in_=d["wqkv"][l, 4 + g])
            for n in range(4):
                ps = ps_gemm.tile([128, S], f32, tag="gemm")
                for c in range(NE):
                    mm(ps, h_t[c][:, n * 128:(n + 1) * 128], wt[:, c, :],
                       start=(c == 0), stop=(c == NE - 1))
                nc.scalar.activation(
                    v_t[n][:, g * 8:(g + 1) * 8, 0:DH],
                    ps.rearrange("p (a b) -> p a b", a=8), AF.Copy)
        # prefetch the first FFN1 weight group (used after attention)
        w1_pre = wpool.tile([128, 8, 512], bf16, tag="w")
        hw.dma_start(out=w1_pre, in_=d["w1"][l, 0])
        # preload the first two head-pairs' masks under the QKV matmuls
        load_masks(0)
        load_masks(1)

    if l == 0 and "dbg_v" in d:
        for n in range(4):
            hw.dma_start(out=d["dbg_v"][n], in_=v_t[n])

    # --- attention ------------------------------------------------------------
    with nc.named_scope(f"L{l}_attn"):
        ctx_pairs = [ctxpool.tile([128, S], bf16, tag="ctx", name=f"cp{l}_{j}")
                     for j in range(8)]
        at_tiles = {}

        # score segments: kc0 -> [0:512], kc1 -> [0:384], kc2+kc3 packed
        # into one psum bank [0:256]+[256:384] (mask cols 896:1280 cover both)
        # masks are loaded into psum FIRST (start=True) so the trailing
        # K=64 score matmuls finish the group pair-concurrently and the
        # exp fires right after them.
        def emit_scores(j):
            qt = qk_t[j]
            kt = qk_t[8 + j]
            for h in (2 * j, 2 * j + 1):
                at_tiles[h] = atpool.tile([128, MASKW], bf16, tag="at",
                                          name=f"at{l}_{h}")
            SEGS = [(0,), (1,), (2, 3)]
            for seg, kcs in enumerate(SEGS):
                off = KOFF[kcs[0]]
                segw = sum(KW[kc] for kc in kcs)
                sps = {}
                for i, h in enumerate((2 * j, 2 * j + 1)):
                    ps = ps_gemm.tile([128, S], f32, tag="gemm",
                                      name=f"s{l}_{h}_{seg}")
                    mm(ps[:, 0:segw], id128, mask_sb[h][:, off:off + segw],
                       start=True, stop=False)
                    sps[h] = ps
                for i, h in enumerate((2 * j, 2 * j + 1)):
                    r0 = 64 * i
                    ps = sps[h]
                    o = 0
                    for kc in kcs:
                        w = KW[kc]
                        q0 = kc * 128
                        mm(ps[:, o:o + w], kt[r0:r0 + DH, q0:q0 + 128],
                           qt[r0:r0 + DH, q0:], start=False,
                           stop=(kc == kcs[-1]), skip_group_check=True)
                        o += w
                for h in (2 * j, 2 * j + 1):
                    nc.scalar.activation(at_tiles[h][:, off:off + segw],
                                         sps[h][:, 0:segw], AF.Exp)
            mask_sb.pop(2 * j), mask_sb.pop(2 * j + 1)

        def emit_av(j):
            # per-head AV with the ones column in v_t: denominator lands in
            # psum row 64 for free (M=65 matmuls are N-bound anyway)
            for i, h in enumerate((2 * j, 2 * j + 1)):
                at = at_tiles.pop(h)
                if l == 0 and h < 2 and "dbg_at" in d:
                    hw.dma_start(out=d["dbg_at"][h], in_=at)
                cpool_ps = ps_ctx if (h % 2 == 0) else ps_score
                cps = cpool_ps.tile([DH + 1, S], f32,
                                    tag="ctxps" if h % 2 == 0 else "score",
                                    name=f"c{l}_{h}")
                for kc in range(4):
                    w = KW[kc]
                    mm(cps[:, kc * 128:], v_t[kc][:, h, :],
                       at[:, KOFF[kc]:KOFF[kc] + w],
                       start=(kc == 0), stop=(kc == 3))
                srow = recpool.tile([1, S], f32, tag="recrow",
                                    name=f"sr{l}_{h}", bufs=2)
                nc.vector.tensor_copy(srow, cps[DH:DH + 1, :])
                rec = recpool.tile([1, S], f32, tag="recrow2",
                                   name=f"re{l}_{h}", bufs=2)
                nc.vector.reciprocal_approx_fast(out=rec, in_=srow)
                recR = recpool.tile([DH, S], f32, tag="rec",
                                    name=f"rr{l}_{h}")
                nc.gpsimd.partition_broadcast(recR, rec, channels=DH)
                r0 = i * 64
                nc.vector.tensor_mul(ctx_pairs[j][r0:r0 + 64, :],
                                     cps[0:DH, :], recR)

        for j in range(8):
            if 1 <= j < 7:
                load_masks(j + 1)
            emit_scores(j)
            if j >= 1:
                emit_av(j - 1)
        emit_av(7)
        # HAM warmth tickles through the softmax tail of the last pairs
        for j in (6, 7):
            tickA = ps_score.tile([128, 4], f32, tag="score",
                                  name=f"tk{l}_{j}")
            mm(tickA, env["ones_r1"], ctx_pairs[j][0:1, 0:4],
               start=True, stop=True)

        r1_t = [None] * 8
        def evict_oproj(pss, grp):
            for mi in range(4):
                mt = grp * 4 + mi
                rtmp = tmppool.tile([128, S], bf16, tag="tmp")
                nc.scalar.activation(rtmp, pss[mi], AF.Identity,
                                     bias=bo_pp[:, mt:mt + 1])
                r1 = hpool.tile([128, S], bf16, tag="h")
                nc.vector.tensor_add(r1, rtmp, h_t[mt])
                if l == 0 and "dbg_r1" in d:
                    hw.dma_start(out=d["dbg_r1"][mt], in_=r1)
                r1_t[mt] = r1

        pssA = [ps_gemm.tile([128, S], f32, tag="gemm", name=f"opA{l}_{i}")
                for i in range(4)]
        for hp in range(8):
            for mi in range(4):
                mm(pssA[mi], wo_sb[:, hp, mi, :], ctx_pairs[hp],
                   start=(hp == 0), stop=(hp == 7))
        evict_oproj(pssA, 0)
        st1 = _ln_begin(env, f"ln{l}_1")
        pssB = [ps_gemm.tile([128, S], f32, tag="gemm", name=f"opB{l}_{i}")
                for i in range(4)]
        for hp in range(8):
            for mi in range(4):
                mm(pssB[mi], wo_sb[:, hp, 4 + mi, :], ctx_pairs[hp],
                   start=(hp == 0), stop=(hp == 7))
        # LN1 sums for the already-evicted first half overlap group B
        _ln_sums(nc, env, st1, r1_t, 0, 4)
        evict_oproj(pssB, 1)
        _ln_sums(nc, env, st1, r1_t, 4, 8)

        if l == 0 and "dbg_ctx" in d:
            for j in range(8):
                hw.dma_start(out=d["dbg_ctx"][j], in_=ctx_pairs[j])

    with nc.named_scope(f"L{l}_ln1"):
        h1_t = _ln_finish(nc, d, st1, r1_t, ln_s[0], ln_b[0], env)
        if l == 0 and "dbg_h1" in d:
            for c in range(NE):
                hw.dma_start(out=d["dbg_h1"][c], in_=h1_t[c])

    # --- FFN -----------------------------------------------------------------
    with nc.named_scope(f"L{l}_ffn"):
        ff_t = []
        for g in range(8):
            if g == 0:
                wt = w1_pre  # prefetched during the QKV phase
            else:
                wt = wpool.tile([128, 8, 512], bf16, tag="w")
                hw.dma_start(out=wt, in_=d["w1"][l, g])
            for mi in range(4):
                mt = g * 4 + mi
                ps = ps_gemm.tile([128, S], f32, tag="gemm")
                for c in range(NE):
                    mm(ps, wt[:, c, mi * 128:(mi + 1) * 128], h1_t[c],
                       start=(c == 0), stop=(c == NE - 1))
                ft = ffpool.tile([128, S], bf16, tag="ff")
                nc.scalar.activation(ft, ps, AF.Gelu,
                                     bias=b1_pp[:, mt:mt + 1])
                ff_t.append(ft)
        # preload the natural_log_exp activation table under FFN2's matmuls
        # so LN2's Ln/Exp don't pay the table swap on the critical path
        nl_dmy = smallf.tile([1, 4], f32, tag="sf", name=f"nld{l}")
        nc.scalar.activation(nl_dmy, ff_t[31][0:1, 0:4], AF.Exp)
        r2_t = [None] * NE
        st2 = _ln_begin(env, f"ln{l}_2")
        for half in range(2):
            pss = [ps_gemm.tile([128, S], f32, tag="gemm",
                                name=f"ff2ps{l}_{half}_{i}") for i in range(4)]
            for cg in range(4):
                wt = wpool.tile([128, 8, 512], bf16, tag="w")
                hw.dma_start(out=wt, in_=d["w2"][l, half, cg])
                for c8 in range(8):
                    c = cg * 8 + c8
                    for mi in range(4):
                        mm(pss[mi], wt[:, c8, mi * 128:(mi + 1) * 128], ff_t[c],
                           start=(c == 0), stop=(c == 31))
            if half == 1:
                # LN2 sums for the first half ride the tail of half1's matmuls
                _ln_sums(nc, env, st2, r2_t, 0, 4)
            for mi in range(4):
                mt = half * 4 + mi
                rtmp = tmppool.tile([128, S], bf16, tag="tmp")
                nc.scalar.activation(rtmp, pss[mi], AF.Identity,
                                     bias=b2_pp[:, mt:mt + 1])
                r2 = hpool.tile([128, S], bf16, tag="h")
                nc.vector.tensor_add(r2, rtmp, h1_t[mt])
                r2_t[mt] = r2
        _ln_sums(nc, env, st2, r2_t, 4, 8)
        wq_next = None
        if l + 1 < L:
            # prefetch next layer's first QKV weight group under FFN2
            wq_next = wpool.tile([128, 8, 512], bf16, tag="w")
            hw.dma_start(out=wq_next, in_=d["wqkv"][l + 1, 0])

    with nc.named_scope(f"L{l}_ln2"):
        h2_t = _ln_finish(nc, d, st2, r2_t, ln_s[1], ln_b[1], env)
        if l == 0 and "dbg_h2" in d:
            for c in range(NE):
                hw.dma_start(out=d["dbg_h2"][c], in_=h2_t[c])
    return h2_t, wq_next


def _patch_act_tables():
    """Force Exp/Ln to resolve to the natural_log_exp table set so LN's
    rstd = exp(-0.5*ln(v)) shares the attention-softmax table set (avoids
    2 table loads per layernorm on the critical path)."""
    if _CACHE.get("tables_patched"):
        return
    import concourse.hw_specs as hw_specs
    orig = hw_specs.get_activation_tables

    def patched(arch):
        tabs = orig(arch)
        for name, fns in tabs.items():
            if name != "natural_log_exp_and_others":
                fns.discard(AF.Exp)
                fns.discard(AF.Ln)
        return tabs

    hw_specs.get_activation_tables = patched
    bacc.get_activation_tables = patched
    _CACHE["tables_patched"] = True


def _build():
    if "nc" in _CACHE:
        return _CACHE["nc"]
    from contextlib import ExitStack

    _patch_act_tables()
    nc = bacc.Bacc("TRN2", debug=False)
    d = _declare(nc)
    with tile.TileContext(nc) as tc:
        with ExitStack() as ctx:
            _emit(nc, tc, d, ctx)
    nc.compile()
    _CACHE["nc"] = nc
    return nc


def kernel_internal(inputs, trace=False, trace_kwargs=None):
    shared = _prep_shared(inputs)
    cores = _prep_percore(inputs)
    nc = _build()
    in_maps = []
    for b in range(B):
        m = dict(shared)
        m.update(cores[b])
        in_maps.append(m)
    res = run_bass_kernel_spmd(
        nc, in_maps, core_ids=list(range(B)), trace=trace,
        **(trace_kwargs or {}),
    )
    outs = []
    for b in range(B):
        lo = res.results[b]["logits"]  # [10, 128, 512]
        lo = lo.reshape(NO * 128, S)[:VV * VR].T  # [512, 1200]
        outs.append(lo)
    out = np.stack(outs).astype(np.float32)  # [B, S, 1200]
    return out, res


def kernel(**inputs):
    out, _ = kernel_internal(inputs)
    return out


# revision 54
# speedup vs baseline: 1.0214x; 1.0214x over previous
"""Trainium2 Bass kernel for nn_BaseGenerator (4-layer dense transformer).

Strategy: pure data-parallel over batch (B=8 -> 8 NeuronCores, no
collectives).  Each core runs the full transformer on one batch element.
Activations are kept feature-major [E, S] in bf16 so every GEMM contracts
over the partition dim; PSUM accumulates in fp32.

Optimizations vs the original baseline (1130us -> ~1005us):
  - scores: K=64 matmuls for head pairs run concurrently on PE row groups
    (0,0)/(64,0); causal column trim (q >= kc*128) on scores/mask/exp/AV;
    kc2+kc3 share one psum bank so each head needs only 3 exp ops.
  - mask packed to [H, 128, 1280] (valid causal region only), preloaded
    into psum (start=True) so score matmuls close each accumulation group.
  - out-proj: two heads' ctx packed into one [128, S] tile -> K=128 mms;
    attention gets the full 8-bank psum budget (scores ride the gemm pool).
  - V bias folded into bo on host (bo' = bo + Wo @ bv).
  - LN: sums via M=128 all-ones matmuls (M=1 psum-row accumulation has a
    ~5x HW serialization penalty), rstd = exp(-0.5*ln(v)) with the
    activation-table patch pinning Exp/Ln to the natural_log_exp set
    (avoids ~2.7us table swaps on the LN critical path), broadcasts via
    K=1 matmuls, sum matmuls overlapped into out-proj B / FFN2 half 1.
  - embedding: one K=94 matmul per chunk (val/ring one-hots stacked).
  - weight/mask DMA prefetching across phase boundaries; bf16 logits.
"""

import os
import sys

for _p in ("/opt/trn_rl_repo",):
    if _p not in sys.path:
        sys.path.insert(0, _p)

import ml_dtypes
import numpy as np

import concourse.bass as bass
import concourse.mybir as mybir
import concourse.tile as tile
from concourse import bacc
from concourse.bass_utils import run_bass_kernel_spmd

BF16 = ml_dtypes.bfloat16

L, E, H, F = 4, 1024, 16, 4096
B, S = 8, 512
VV, VR = 40, 30
DIST_V = 200
PAD_ID = 0
DH = E // H  # 64
NE = E // 128  # 8 feature chunks
NO = 10  # logit row tiles (1280 padded)
NEG = -1.0e9

# causal column layout: for k-chunk kc, valid q range is [kc*128, 512)
KOFF = [0, 512, 896, 1152]
KW = [512, 384, 256, 128]
MASKW = 1280

f32 = mybir.dt.float32
bf16 = mybir.dt.bfloat16
AF = mybir.ActivationFunctionType
OP = mybir.AluOpType

_CACHE = {}


# ----------------------------------------------------------------------------
# host-side input prep
# ----------------------------------------------------------------------------

def _prep_shared(inp):
    """Weight-layout prep shared by all cores. Returns dict name->np array."""
    out = {}

    def b16(x):
        return np.ascontiguousarray(x.astype(BF16))

    Wqkv = np.asarray(inp["Wqkv"], np.float32).copy()  # [L, 3E, E]
    bqkv = np.asarray(inp["bqkv"], np.float32).copy()  # [L, 3E]
    bv = bqkv[:, 2 * E:].copy()  # [L, E] (V bias, folded into bo below)
    # fold attention scale into Q projection
    scale = 1.0 / np.sqrt(DH)
    Wqkv[:, :E, :] *= scale
    bqkv[:, :E] *= scale

    def block_lhsT(W, gsize):
        # W: [L?, OUT, IN] -> [.., G, 128, IN//128, gsize] with
        # out[..., g, p, c, o] = W[..., g*gsize + o, c*128 + p]
        *lead, O, I = W.shape
        G = O // gsize
        nc_ = I // 128
        Wb = W.reshape(*lead, G, gsize, nc_, 128)
        Wb = np.moveaxis(Wb, -1, -3)  # [..., G, 128, gsize, nc]
        Wb = np.swapaxes(Wb, -1, -2)  # [..., G, 128, nc, gsize]
        return np.ascontiguousarray(Wb)

    out["wqkv"] = b16(block_lhsT(Wqkv, 512))          # [L, 6, 128, 8, 512]

    # Wo head-pair major: wo2[l, p, hp, mt, o] = Wo[l, mt*128+o, hp*128+p]
    Wo = np.asarray(inp["Wo"], np.float32)  # [L, E(out), E(in=ctx)]
    t = Wo.reshape(L, 8, 128, 8, 128)       # [l, mt, o, hp, p]
    out["wo2"] = b16(t.transpose(0, 4, 3, 1, 2))  # [l, p, hp, mt, o]

    # bo' = bo + Wo @ bv  (ctx rows are normalized, sum of attn weights = 1)
    bo2 = np.asarray(inp["bo"], np.float32) + np.einsum("loi,li->lo", Wo, bv)

    out["w1"] = b16(block_lhsT(np.asarray(inp["W1"], np.float32), 512))  # [L,8,128,8,512]
    W2 = np.asarray(inp["W2"], np.float32)  # out=E, in=F
    w2b = block_lhsT(W2, 512)  # [L, 2, 128, 32, 512]
    w2b = w2b.reshape(L, 2, 128, 4, 8, 512).transpose(0, 1, 3, 2, 4, 5)
    out["w2"] = b16(w2b)  # [L, 2, 4, 128, 8, 512]

    genW = np.asarray(inp["gen_W"], np.float32)  # [1200, E]
    genW_pad = np.zeros((1280, E), np.float32)
    genW_pad[:1200] = genW
    out["genw"] = b16(block_lhsT(genW_pad, 640))  # [2, 128, 8, 640]

    gen_b = np.asarray(inp["gen_b"], np.float32)
    gbp = np.zeros((1280,), np.float32)
    gbp[:1200] = gen_b
    out["gen_b_pp"] = np.ascontiguousarray(gbp.reshape(NO, 128).T)  # [128, 10]

    def pp(v):  # [..., N*128] -> [..., 128, N]
        *lead, N = v.shape
        return np.ascontiguousarray(
            v.reshape(*lead, N // 128, 128).swapaxes(-1, -2).astype(np.float32)
        )

    out["bqkv_pp"] = pp(bqkv[:, : 2 * E])  # [L, 128, 16] (Q scaled)
    out["bo_pp"] = pp(bo2)  # [L, 128, 8]
    out["b1_pp"] = pp(np.asarray(inp["b1"], np.float32))  # [L, 128, 32]
    out["b2_pp"] = pp(np.asarray(inp["b2"], np.float32))  # [L, 128, 8]

    ln_s = np.stack([np.asarray(inp["ln1_s"], np.float32),
                     np.asarray(inp["ln2_s"], np.float32)], 1)  # [L, 2, E]
    ln_b = np.stack([np.asarray(inp["ln1_b"], np.float32),
                     np.asarray(inp["ln2_b"], np.float32)], 1)
    out["ln_s_pp"] = pp(ln_s)  # [L, 2, 128, 8]
    out["ln_b_pp"] = pp(ln_b)
    out["lnf_s_pp"] = pp(np.asarray(inp["lnf_s"], np.float32))  # [128, 8]
    out["lnf_b_pp"] = pp(np.asarray(inp["lnf_b"], np.float32))

    # stacked embedding table: rows 0:40 val, 40:64 zero, 64:94 ring
    embcat = np.zeros((94, E), np.float32)
    embcat[0:VV] = np.asarray(inp["val_emb"], np.float32)
    embcat[64:64 + VR] = np.asarray(inp["ring_emb"], np.float32)
    out["embcat"] = b16(embcat)

    # iota for the stacked one-hot: 0..39 | -1 x24 | 0..29 | -1 x34
    iota94 = np.full((128, 1), -1.0, np.float32)
    iota94[0:VV, 0] = np.arange(VV)
    iota94[64:64 + VR, 0] = np.arange(VR)
    out["iota94"] = np.ascontiguousarray(iota94)

    out["id128"] = b16(np.eye(128, dtype=np.float32))
    out["ones_col"] = b16(np.ones((128, 1), np.float32))
    return out


def _prep_percore(inp):
    """Per-core tensors: token rows + packed causal attention mask."""
    val = np.asarray(inp["val_sequences"]).astype(np.int64)    # [B, S]
    ring = np.asarray(inp["ring_sequences"]).astype(np.int64)  # [B, S]
    dist = np.asarray(inp["distance_squares"]).astype(np.int64)  # [B, S, S]
    de = np.asarray(inp["dist_emb"], np.float32)  # [200, H]

    # mask[b, h, k, q] = de[dist[b, q, k], h] or NEG
    m = de[dist]                         # [B, S(q), S(k), H]
    m = m.transpose(0, 3, 2, 1)          # [B, H, k, q]
    kk = np.arange(S)
    causal = kk[:, None] <= kk[None, :]  # [k, q] keep where k <= q
    m = np.where(causal[None, None], m, NEG)
    padk = val == PAD_ID  # [B, S]
    m = np.where(padk[:, None, :, None], NEG, m)
    # pack causal region: [B, H, 128, 1280]; chunk kc covers q in [kc*128,512)
    mp = np.empty((B, H, 128, MASKW), np.float32)
    for kc in range(4):
        mp[:, :, :, KOFF[kc]:KOFF[kc] + KW[kc]] = (
            m[:, :, kc * 128:(kc + 1) * 128, kc * 128:]
        )
    mp = np.ascontiguousarray(mp.astype(BF16))

    cores = []
    for b in range(B):
        cores.append({
            "mask": mp[b],
            "valrow": np.ascontiguousarray(val[b].reshape(1, S).astype(BF16)),
            "ringrow": np.ascontiguousarray(ring[b].reshape(1, S).astype(BF16)),
        })
    return cores


# ----------------------------------------------------------------------------
# device program
# ----------------------------------------------------------------------------

def _declare(nc):
    d = {}

    def di(name, shape, dt):
        d[name] = nc.dram_tensor(name, list(shape), dt, kind="ExternalInput").ap()

    di("wqkv", (L, 6, 128, 8, 512), bf16)
    di("wo2", (L, 128, 8, 8, 128), bf16)
    di("w1", (L, 8, 128, 8, 512), bf16)
    di("w2", (L, 2, 4, 128, 8, 512), bf16)
    di("genw", (2, 128, 8, 640), bf16)
    di("gen_b_pp", (128, NO), f32)
    di("bqkv_pp", (L, 128, 16), f32)
    di("bo_pp", (L, 128, 8), f32)
    di("b1_pp", (L, 128, 32), f32)
    di("b2_pp", (L, 128, 8), f32)
    di("ln_s_pp", (L, 2, 128, 8), f32)
    di("ln_b_pp", (L, 2, 128, 8), f32)
    di("lnf_s_pp", (128, 8), f32)
    di("lnf_b_pp", (128, 8), f32)
    di("embcat", (94, E), bf16)
    di("iota94", (128, 1), f32)
    di("id128", (128, 128), bf16)
    di("ones_col", (128, 1), bf16)
    di("mask", (H, 128, MASKW), bf16)
    di("valrow", (1, S), bf16)
    di("ringrow", (1, S), bf16)
    d["logits"] = nc.dram_tensor(
        "logits", [NO, 128, S], bf16, kind="ExternalOutput"
    ).ap()
    if os.environ.get("BG_DEBUG"):
        def do(name, shape):
            d[name] = nc.dram_tensor(name, list(shape), bf16,
                                     kind="ExternalOutput").ap()
        do("dbg_h0", (NE, 128, S))
        do("dbg_qk", (16, 128, S))
        do("dbg_v", (4, 128, H, DH + 1))
        do("dbg_at", (2, 128, MASKW))
        do("dbg_ctx", (8, 128, S))
        do("dbg_r1", (NE, 128, S))
        do("dbg_h1", (NE, 128, S))
        do("dbg_h2", (NE, 128, S))
    return d


def _emit(nc, tc, d, ctx):
    mm = nc.tensor.matmul

    cpool = ctx.enter_context(tc.tile_pool(name="cpool", bufs=1))
    wpool = ctx.enter_context(tc.tile_pool(name="wpool", bufs=4))
    wopool = ctx.enter_context(tc.tile_pool(name="wopool", bufs=1))
    hpool = ctx.enter_context(tc.tile_pool(name="hpool", bufs=17))
    qkpool = ctx.enter_context(tc.tile_pool(name="qkpool", bufs=16))
    vpool = ctx.enter_context(tc.tile_pool(name="vpool", bufs=5))
    maskpool = ctx.enter_context(tc.tile_pool(name="maskpool", bufs=4))
    atpool = ctx.enter_context(tc.tile_pool(name="atpool", bufs=4))
    ctxpool = ctx.enter_context(tc.tile_pool(name="ctxpool", bufs=9))
    ffpool = ctx.enter_context(tc.tile_pool(name="ffpool", bufs=33))
    tmppool = ctx.enter_context(tc.tile_pool(name="tmppool", bufs=4))
    smallf = ctx.enter_context(tc.tile_pool(name="smallf", bufs=5))
    smallb = ctx.enter_context(tc.tile_pool(name="smallb", bufs=4))
    recpool = ctx.enter_context(tc.tile_pool(name="recpool", bufs=3))
    lnbpool = ctx.enter_context(tc.tile_pool(name="lnbpool", bufs=4))
    outpool = ctx.enter_context(tc.tile_pool(name="outpool", bufs=2))
    pppool = ctx.enter_context(tc.tile_pool(name="pppool", bufs=4))

    ps_gemm = ctx.enter_context(tc.tile_pool(name="ps_gemm", bufs=4, space="PSUM"))
    ps_score = ctx.enter_context(tc.tile_pool(name="ps_score", bufs=2, space="PSUM"))
    ps_ctx = ctx.enter_context(tc.tile_pool(name="ps_ctx", bufs=2, space="PSUM"))

    hw = nc.sync  # HWDGE dma engine

    # --- constants -----------------------------------------------------------
    id128 = cpool.tile([128, 128], bf16)
    hw.dma_start(out=id128, in_=d["id128"])
    ones_col = cpool.tile([128, 1], bf16)
    hw.dma_start(out=ones_col, in_=d["ones_col"])
    iota94 = cpool.tile([128, 1], f32)
    hw.dma_start(out=iota94, in_=d["iota94"])
    embcat = cpool.tile([94, E], bf16)
    hw.dma_start(out=embcat, in_=d["embcat"])
    genb_pp = cpool.tile([128, NO], f32)
    hw.dma_start(out=genb_pp, in_=d["gen_b_pp"])
    lnf_s = cpool.tile([128, 8], f32)
    hw.dma_start(out=lnf_s, in_=d["lnf_s_pp"])
    lnf_b = cpool.tile([128, 8], f32)
    hw.dma_start(out=lnf_b, in_=d["lnf_b_pp"])
    eps_t = cpool.tile([128, 1], f32)
    nc.vector.memset(eps_t, 1e-5)
    ones_r1 = cpool.tile([1, 128], bf16)
    nc.vector.memset(ones_r1, 1.0)
    ones128 = cpool.tile([128, 128], bf16)
    nc.vector.memset(ones128, 1.0)
    # prefetch first QKV weight group of layer 0 (hides DMA cold-start)
    wq_pre = wpool.tile([128, 8, 512], bf16, tag="w")
    hw.dma_start(out=wq_pre, in_=d["wqkv"][0, 0])

    # --- embedding -----------------------------------------------------------
    with nc.named_scope("embed"):
        vr = tmppool.tile([94, S], bf16, tag="sq")
        nc.vector.memset(vr[32:64, :], -2.0)
        nc.gpsimd.dma_start(out=vr[0:VV, :], in_=d["valrow"].to_broadcast((VV, S)))
        nc.gpsimd.dma_start(out=vr[64:64 + VR, :],
                            in_=d["ringrow"].to_broadcast((VR, S)))
        oh = tmppool.tile([94, S], bf16, tag="tmp")
        nc.vector.tensor_scalar(oh, vr, iota94[0:94, :], None, OP.is_equal)

        h_t = []
        for c in range(NE):
            ps = ps_gemm.tile([128, S], f32, tag="gemm")
            mm(ps, embcat[:, c * 128:(c + 1) * 128], oh, start=True, stop=True)
            ht = hpool.tile([128, S], bf16, tag="h")
            nc.scalar.activation(ht, ps, AF.Copy, scale=float(np.sqrt(E)))
            if "dbg_h0" in d:
                hw.dma_start(out=d["dbg_h0"][c], in_=ht)
            h_t.append(ht)

    # --- layers --------------------------------------------------------------
    env = dict(locals())
    for l in range(L):
        h_t, wq_pre = _layer(nc, tc, d, l, h_t, env, wq_pre)

    # --- final LN + head -----------------------------------------------------
    with nc.named_scope("final"):
        genw_sb = []
        for g in range(2):
            wt = wpool.tile([128, 8, 640], bf16, tag="w")
            hw.dma_start(out=wt, in_=d["genw"][g])
            genw_sb.append(wt)
        hf = _layernorm(nc, d, h_t, lnf_s, lnf_b, env, "lnf")
        for mt in range(NO):
            g, mi = divmod(mt, 5)
            ps = ps_gemm.tile([128, S], f32, tag="gemm")
            for c in range(NE):
                mm(ps, genw_sb[g][:, c, mi * 128:(mi + 1) * 128], hf[c],
                   start=(c == 0), stop=(c == NE - 1))
            ot = outpool.tile([128, S], bf16, tag="bfout")
            nc.scalar.activation(ot, ps, AF.Identity, bias=genb_pp[:, mt:mt + 1])
            hw.dma_start(out=d["logits"][mt], in_=ot)


def _layernorm(nc, d, r_t, s_pp, b_pp, env, nm):
    """r_t: 8 bf16 [128, S] feature-major tiles -> returns normalized tiles.

    sums via 2-way col-tiled M=1 matmuls; rstd = exp(-0.5 * ln(var + eps))
    to stay inside the natural_log_exp activation table set.
    """
    st = _ln_begin(env, nm)
    _ln_sums(nc, env, st, r_t, 0, NE)
    return _ln_finish(nc, d, st, r_t, s_pp, b_pp, env)


def _ln_begin(env, nm):
    """Allocate the two psum stat accumulators (no instructions)."""
    ps_ctx = env["ps_ctx"]
    psS = ps_ctx.tile([128, S], f32, tag="ctxps", name=f"{nm}_psS")
    psQ = ps_ctx.tile([128, S], f32, tag="ctxps", name=f"{nm}_psQ")
    return {"psS": psS, "psQ": psQ, "nm": nm}


def _ln_sums(nc, env, st, r_t, c0, c1):
    """Emit sum/sumsq matmuls for chunks [c0, c1). M=128 all-ones lhsT:
    one matmul per chunk at full rate (M=1 accumulation into a single
    psum row has a ~5x serialization penalty on HW); every psum partition
    holds the full sum."""
    mm = nc.tensor.matmul
    tmppool = env["tmppool"]
    for c in range(c0, c1):
        sq = tmppool.tile([128, S], bf16, tag="sq", name=f"{st['nm']}_sq{c}")
        nc.vector.tensor_mul(sq, r_t[c], r_t[c])
        mm(st["psS"], env["ones128"], r_t[c],
           start=(c == 0), stop=(c == NE - 1))
        mm(st["psQ"], env["ones128"], sq,
           start=(c == 0), stop=(c == NE - 1))


def _ln_finish(nc, d, st, r_t, s_pp, b_pp, env):
    mm = nc.tensor.matmul
    ps_score = env["ps_score"]
    smallf = env["smallf"]; smallb = env["smallb"]
    tmppool = env["tmppool"]; hpool = env["hpool"]; lnbpool = env["lnbpool"]
    psS, psQ, nm = st["psS"], st["psQ"], st["nm"]

    s2 = smallf.tile([1, S], f32, tag="sf", name=f"{nm}_s2")
    nc.scalar.activation(s2, psS[0:1, :], AF.Square)
    varE = smallf.tile([1, S], f32, tag="sf", name=f"{nm}_varE")
    # varE = sumsq - s2/E  (= E * var)
    nc.vector.scalar_tensor_tensor(varE, s2, -1.0 / E, psQ[0:1, :],
                                   OP.mult, OP.add)
    # HAM warmth tickle mid-chain (depends on varE via a tiny bf16 copy)
    vb = smallb.tile([1, 4], bf16, tag="vb")
    nc.vector.tensor_copy(vb, varE[:, 0:4])
    tick = ps_score.tile([128, 4], f32, tag="score", name=f"{nm}_t0")
    mm(tick, env["ones_r1"], vb, start=True, stop=True)

    lnv = smallf.tile([1, S], f32, tag="sf", name=f"{nm}_lnv")
    nc.scalar.activation(lnv, varE, AF.Ln, scale=1.0 / E,
                         bias=env["eps_t"][0:1, :])
    rstd = smallf.tile([1, S], f32, tag="sf", name=f"{nm}_rstd")
    nc.scalar.activation(rstd, lnv, AF.Exp, scale=-0.5)

    rstd_b = smallb.tile([1, S], bf16, tag="sb")
    nc.vector.tensor_copy(rstd_b, rstd)
    u_b = smallb.tile([1, S], bf16, tag="sb")
    # u = mean * rstd = (sum/E) * rstd
    nc.vector.scalar_tensor_tensor(u_b, psS[0:1, :], 1.0 / E, rstd,
                                   OP.mult, OP.mult)

    # broadcast rstd / u to 128 partitions with K=1 matmuls, then copy to
    # SBUF bf16 so the tail DVE ops run at 2x 16-bit rate
    rstdR_ps = ps_score.tile([128, S], f32, tag="score", name=f"{nm}_rstdRp")
    mm(rstdR_ps, env["ones_r1"], rstd_b, start=True, stop=True)
    uR_ps = ps_score.tile([128, S], f32, tag="score", name=f"{nm}_uRp")
    mm(uR_ps, env["ones_r1"], u_b, start=True, stop=True)
    rstdR = lnbpool.tile([128, S], bf16, tag="lnb", name=f"{nm}_rstdR")
    nc.scalar.activation(rstdR, rstdR_ps, AF.Copy)
    uR = lnbpool.tile([128, S], bf16, tag="lnb", name=f"{nm}_uR")
    nc.vector.tensor_copy(uR, uR_ps)

    out_t = []
    for c in range(NE):
        t1 = tmppool.tile([128, S], bf16, tag="tmp")
        nc.vector.tensor_mul(t1, r_t[c], rstdR)
        t2 = tmppool.tile([128, S], bf16, tag="sq")
        nc.vector.tensor_sub(t2, t1, uR)
        ht = hpool.tile([128, S], bf16, tag="h")
        nc.scalar.activation(ht, t2, AF.Identity,
                             bias=b_pp[:, c:c + 1], scale=s_pp[:, c:c + 1])
        out_t.append(ht)
    return out_t


def _layer(nc, tc, d, l, h_t, env, wq0):
    mm = nc.tensor.matmul
    hw = nc.sync
    wpool = env["wpool"]; wopool = env["wopool"]; hpool = env["hpool"]
    qkpool = env["qkpool"]; vpool = env["vpool"]; maskpool = env["maskpool"]
    atpool = env["atpool"]; ctxpool = env["ctxpool"]; ffpool = env["ffpool"]
    tmppool = env["tmppool"]; smallf = env["smallf"]; recpool = env["recpool"]
    pppool = env["pppool"]
    ps_gemm = env["ps_gemm"]; ps_score = env["ps_score"]; ps_ctx = env["ps_ctx"]
    id128 = env["id128"]; ones_col = env["ones_col"]

    # per-layer small params
    bqkv_pp = pppool.tile([128, 16], f32, tag="pp16")
    hw.dma_start(out=bqkv_pp, in_=d["bqkv_pp"][l])
    bo_pp = pppool.tile([128, 8], f32, tag="pp8")
    hw.dma_start(out=bo_pp, in_=d["bo_pp"][l])
    b1_pp = pppool.tile([128, 32], f32, tag="pp32")
    hw.dma_start(out=b1_pp, in_=d["b1_pp"][l])
    b2_pp = pppool.tile([128, 8], f32, tag="pp8")
    hw.dma_start(out=b2_pp, in_=d["b2_pp"][l])
    ln_s = [pppool.tile([128, 8], f32, tag="pp8", name=f"lns{l}_{i}")
            for i in range(2)]
    ln_b = [pppool.tile([128, 8], f32, tag="pp8", name=f"lnb{l}_{i}")
            for i in range(2)]
    for i in range(2):
        hw.dma_start(out=ln_s[i], in_=d["ln_s_pp"][l, i])
        hw.dma_start(out=ln_b[i], in_=d["ln_b_pp"][l, i])
    mask_sb = {}

    def load_masks(j):
        for h in (2 * j, 2 * j + 1):
            mk = maskpool.tile([128, MASKW], bf16, tag="mask",
                               name=f"mk{l}_{h}")
            hw.dma_start(out=mk, in_=d["mask"][h])
            mask_sb[h] = mk

    # --- QKV -----------------------------------------------------------------
    def emit_qk_chunk(wt, g, mi):
        mt = g * 4 + mi
        ps = ps_gemm.tile([128, S], f32, tag="gemm", name=f"qk{l}_{mt}")
        for c in range(NE):
            mm(ps, wt[:, c, mi * 128:(mi + 1) * 128], h_t[c],
               start=(c == 0), stop=(c == NE - 1))
        qk = qkpool.tile([128, S], bf16, tag="qk", name=f"qkt{l}_{mt}")
        nc.scalar.activation(qk, ps, AF.Identity, bias=bqkv_pp[:, mt:mt + 1])
        if l == 0 and "dbg_qk" in d:
            hw.dma_start(out=d["dbg_qk"][mt], in_=qk)
        qk_t[(g // 2) * 8 + (g % 2) * 4 + mi] = qk

    with nc.named_scope(f"L{l}_qkv"):
        qk_t = [None] * 16  # q 0..7, k 8..15
        # first half of Q and K (chunks 0-3) so attention can start early
        wtK0 = wpool.tile([128, 8, 512], bf16, tag="w")
        hw.dma_start(out=wtK0, in_=d["wqkv"][l, 2])
        wo_sb = wopool.tile([128, 8, 8, 128], bf16, tag="wo")
        hw.dma_start(out=wo_sb, in_=d["wo2"][l])
        for mi in range(4):
            emit_qk_chunk(wq0, 0, mi)
        for mi in range(4):
            emit_qk_chunk(wtK0, 2, mi)
        # second-half weights (their chunk GEMMs are interleaved into the
        # first attention pairs to fill the exp-paced stalls)
        wtQ1 = wpool.tile([128, 8, 512], bf16, tag="w")
        hw.dma_start(out=wtQ1, in_=d["wqkv"][l, 1])
        wtK1 = wpool.tile([128, 8, 512], bf16, tag="w")
        hw.dma_start(out=wtK1, in_=d["wqkv"][l, 3])
        # V token-major [128, H, DH+1] with ones column (softmax denominator)
        v_t = []
        for n in range(4):
            vt = vpool.tile([128, H, DH + 1], bf16, tag="v", name=f"v{l}_{n}")
            nc.vector.memset(vt[:, :, DH:DH + 1], 1.0)
            v_t.append(vt)
        for g in range(2):
            wt = wpool.tile([128, 8, 512], bf16, tag="w")
            hw.dma_start(out=wt, in_=d["wqkv"][l, 4 + g])
            for n in range(4):
                ps = ps_gemm.tile([128, S], f32, tag="gemm")
                for c in range(NE):
                    mm(ps, h_t[c][:, n * 128:(n + 1) * 128], wt[:, c, :],
                       start=(c == 0), stop=(c == NE - 1))
                nc.scalar.activation(
                    v_t[n][:, g * 8:(g + 1) * 8, 0:DH],
                    ps.rearrange("p (a b) -> p a b", a=8), AF.Copy)
        # prefetch the first FFN1 weight group (used after attention)
        w1_pre = wpool.tile([128, 8, 512], bf16, tag="w")
        hw.dma_start(out=w1_pre, in_=d["w1"][l, 0])
        # preload the first two head-pairs' masks under the QKV matmuls
        load_masks(0)
        load_masks(1)

    if l == 0 and "dbg_v" in d:
        for n in range(4):
            hw.dma_start(out=d["dbg_v"][n], in_=v_t[n])

    # --- attention ------------------------------------------------------------
    with nc.named_scope(f"L{l}_attn"):
        ctx_pairs = [ctxpool.tile([128, S], bf16, tag="ctx", name=f"cp{l}_{j}")
                     for j in range(8)]
        at_tiles = {}

        # score segments: kc0 -> [0:512], kc1 -> [0:384], kc2+kc3 packed
        # into one psum bank [0:256]+[256:384] (mask cols 896:1280 cover both)
        # masks are loaded into psum FIRST (start=True) so the trailing
        # K=64 score matmuls finish the group pair-concurrently and the
        # exp fires right after them.
        def emit_scores(j):
            qt = qk_t[j]
            kt = qk_t[8 + j]
            for h in (2 * j, 2 * j + 1):
                at_tiles[h] = atpool.tile([128, MASKW], bf16, tag="at",
                                          name=f"at{l}_{h}")
            SEGS = [(0,), (1,), (2, 3)]
            for seg, kcs in enumerate(SEGS):
                off = KOFF[kcs[0]]
                segw = sum(KW[kc] for kc in kcs)
                sps = {}
                for i, h in enumerate((2 * j, 2 * j + 1)):
                    ps = ps_gemm.tile([128, S], f32, tag="gemm",
                                      name=f"s{l}_{h}_{seg}")
                    mm(ps[:, 0:segw], id128, mask_sb[h][:, off:off + segw],
                       start=True, stop=False)
                    sps[h] = ps
                for i, h in enumerate((2 * j, 2 * j + 1)):
                    r0 = 64 * i
                    ps = sps[h]
                    o = 0
                    for kc in kcs:
                        w = KW[kc]
                        q0 = kc * 128
                        mm(ps[:, o:o + w], kt[r0:r0 + DH, q0:q0 + 128],
                           qt[r0:r0 + DH, q0:], start=False,
                           stop=(kc == kcs[-1]), skip_group_check=True)
                        o += w
                for h in (2 * j, 2 * j + 1):
                    nc.scalar.activation(at_tiles[h][:, off:off + segw],
                                         sps[h][:, 0:segw], AF.Exp)
            mask_sb.pop(2 * j), mask_sb.pop(2 * j + 1)

        def emit_av(j):
            # per-head AV with the ones column in v_t: denominator lands in
            # psum row 64 for free (M=65 matmuls are N-bound anyway)
            for i, h in enumerate((2 * j, 2 * j + 1)):
                at = at_tiles.pop(h)
                if l == 0 and h < 2 and "dbg_at" in d:
                    hw.dma_start(out=d["dbg_at"][h], in_=at)
                cpool_ps = ps_ctx if (h % 2 == 0) else ps_score
                cps = cpool_ps.tile([DH + 1, S], f32,
                                    tag="ctxps" if h % 2 == 0 else "score",
                                    name=f"c{l}_{h}")
                for kc in range(4):
                    w = KW[kc]
                    mm(cps[:, kc * 128:], v_t[kc][:, h, :],
                       at[:, KOFF[kc]:KOFF[kc] + w],
                       start=(kc == 0), stop=(kc == 3))
                srow = recpool.tile([1, S], f32, tag="recrow",
                                    name=f"sr{l}_{h}", bufs=2)
                nc.vector.tensor_copy(srow, cps[DH:DH + 1, :])
                rec = recpool.tile([1, S], f32, tag="recrow2",
                                   name=f"re{l}_{h}", bufs=2)
                nc.vector.reciprocal_approx_fast(out=rec, in_=srow)
                recR = recpool.tile([DH, S], f32, tag="rec",
                                    name=f"rr{l}_{h}")
                nc.gpsimd.partition_broadcast(recR, rec, channels=DH)
                r0 = i * 64
                nc.vector.tensor_mul(ctx_pairs[j][r0:r0 + 64, :],
                                     cps[0:DH, :], recR)

        for j in range(8):
            if 1 <= j < 7:
                load_masks(j + 1)
            emit_scores(j)
            if j < 4:
                emit_qk_chunk(wtQ1, 1, j)
                emit_qk_chunk(wtK1, 3, j)
            if j >= 1:
                emit_av(j - 1)
        emit_av(7)
        # HAM warmth tickles through the softmax tail of the last pairs
        for j in (6, 7):
            tickA = ps_score.tile([128, 4], f32, tag="score",
                                  name=f"tk{l}_{j}")
            mm(tickA, env["ones_r1"], ctx_pairs[j][0:1, 0:4],
               start=True, stop=True)

        r1_t = [None] * 8
        def evict_oproj(pss, grp):
            for mi in range(4):
                mt = grp * 4 + mi
                rtmp = tmppool.tile([128, S], bf16, tag="tmp")
                nc.scalar.activation(rtmp, pss[mi], AF.Identity,
                                     bias=bo_pp[:, mt:mt + 1])
                r1 = hpool.tile([128, S], bf16, tag="h")
                nc.vector.tensor_add(r1, rtmp, h_t[mt])
                if l == 0 and "dbg_r1" in d:
                    hw.dma_start(out=d["dbg_r1"][mt], in_=r1)
                r1_t[mt] = r1

        pssA = [ps_gemm.tile([128, S], f32, tag="gemm", name=f"opA{l}_{i}")
                for i in range(4)]
        for hp in range(8):
            for mi in range(4):
                mm(pssA[mi], wo_sb[:, hp, mi, :], ctx_pairs[hp],
                   start=(hp == 0), stop=(hp == 7))
        evict_oproj(pssA, 0)
        st1 = _ln_begin(env, f"ln{l}_1")
        pssB = [ps_gemm.tile([128, S], f32, tag="gemm", name=f"opB{l}_{i}")
                for i in range(4)]
        for hp in range(8):
            for mi in range(4):
                mm(pssB[mi], wo_sb[:, hp, 4 + mi, :], ctx_pairs[hp],
                   start=(hp == 0), stop=(hp == 7))
        # LN1 sums for the already-evicted first half overlap group B
        _ln_sums(nc, env, st1, r1_t, 0, 4)
        evict_oproj(pssB, 1)
        _ln_sums(nc, env, st1, r1_t, 4, 8)

        if l == 0 and "dbg_ctx" in d:
            for j in range(8):
                hw.dma_start(out=d["dbg_ctx"][j], in_=ctx_pairs[j])

    with nc.named_scope(f"L{l}_ln1"):
        h1_t = _ln_finish(nc, d, st1, r1_t, ln_s[0], ln_b[0], env)
        if l == 0 and "dbg_h1" in d:
            for c in range(NE):
                hw.dma_start(out=d["dbg_h1"][c], in_=h1_t[c])

    # --- FFN -----------------------------------------------------------------
    with nc.named_scope(f"L{l}_ffn"):
        ff_t = []
        for g in range(8):
            if g == 0:
                wt = w1_pre  # prefetched during the QKV phase
            else:
                wt = wpool.tile([128, 8, 512], bf16, tag="w")
                hw.dma_start(out=wt, in_=d["w1"][l, g])
            for mi in range(4):
                mt = g * 4 + mi
                ps = ps_gemm.tile([128, S], f32, tag="gemm")
                for c in range(NE):
                    mm(ps, wt[:, c, mi * 128:(mi + 1) * 128], h1_t[c],
                       start=(c == 0), stop=(c == NE - 1))
                ft = ffpool.tile([128, S], bf16, tag="ff")
                nc.scalar.activation(ft, ps, AF.Gelu,
                                     bias=b1_pp[:, mt:mt + 1])
                ff_t.append(ft)
        # preload the natural_log_exp activation table under FFN2's matmuls
        # so LN2's Ln/Exp don't pay the table swap on the critical path
        nl_dmy = smallf.tile([1, 4], f32, tag="sf", name=f"nld{l}")
        nc.scalar.activation(nl_dmy, ff_t[31][0:1, 0:4], AF.Exp)
        r2_t = [None] * NE
        st2 = _ln_begin(env, f"ln{l}_2")
        for half in range(2):
            pss = [ps_gemm.tile([128, S], f32, tag="gemm",
                                name=f"ff2ps{l}_{half}_{i}") for i in range(4)]
            for cg in range(4):
                wt = wpool.tile([128, 8, 512], bf16, tag="w")
                hw.dma_start(out=wt, in_=d["w2"][l, half, cg])
                for c8 in range(8):
                    c = cg * 8 + c8
                    for mi in range(4):
                        mm(pss[mi], wt[:, c8, mi * 128:(mi + 1) * 128], ff_t[c],
                           start=(c == 0), stop=(c == 31))
            if half == 1:
                # LN2 sums for the first half ride the tail of half1's matmuls
                _ln_sums(nc, env, st2, r2_t, 0, 4)
            for mi in range(4):
                mt = half * 4 + mi
                rtmp = tmppool.tile([128, S], bf16, tag="tmp")
                nc.scalar.activation(rtmp, pss[mi], AF.Identity,
                                     bias=b2_pp[:, mt:mt + 1])
                r2 = hpool.tile([128, S], bf16, tag="h")
                nc.vector.tensor_add(r2, rtmp, h1_t[mt])
                r2_t[mt] = r2
        _ln_sums(nc, env, st2, r2_t, 4, 8)
        wq_next = None
        if l + 1 < L:
            # prefetch next layer's first QKV weight group under FFN2
            wq_next = wpool.tile([128, 8, 512], bf16, tag="w")
            hw.dma_start(out=wq_next, in_=d["wqkv"][l + 1, 0])

    with nc.named_scope(f"L{l}_ln2"):
        h2_t = _ln_finish(nc, d, st2, r2_t, ln_s[1], ln_b[1], env)
        if l == 0 and "dbg_h2" in d:
            for c in range(NE):
                hw.dma_start(out=d["dbg_h2"][c], in_=h2_t[c])
    return h2_t, wq_next


def _patch_act_tables():
    """Force Exp/Ln to resolve to the natural_log_exp table set so LN's
    rstd = exp(-0.5*ln(v)) shares the attention-softmax table set (avoids
    2 table loads per layernorm on the critical path)."""
    if _CACHE.get("tables_patched"):
        return
    import concourse.hw_specs as hw_specs
    orig = hw_specs.get_activation_tables

    def patched(arch):
        tabs = orig(arch)
        for name, fns in tabs.items():
            if name != "natural_log_exp_and_others":
                fns.discard(AF.Exp)
                fns.discard(AF.Ln)
        return tabs

    hw_specs.get_activation_tables = patched
    bacc.get_activation_tables = patched
    _CACHE["tables_patched"] = True


def _build():
    if "nc" in _CACHE:
        return _CACHE["nc"]
    from contextlib import ExitStack

    _patch_act_tables()
    nc = bacc.Bacc("TRN2", debug=False)
    d = _declare(nc)
    with tile.TileContext(nc) as tc:
        with ExitStack() as ctx:
            _emit(nc, tc, d, ctx)
    nc.compile()
    _CACHE["nc"] = nc
    return nc


def kernel_internal(inputs, trace=False, trace_kwargs=None):
    shared = _prep_shared(inputs)
    cores = _prep_percore(inputs)
    nc = _build()
    in_maps = []
    for b in range(B):
        m = dict(shared)
        m.update(cores[b])
        in_maps.append(m)
    res = run_bass_kernel_spmd(
        nc, in_maps, core_ids=list(range(B)), trace=trace,
        **(trace_kwargs or {}),
    )
    outs = []
    for b in range(B):
        lo = res.results[b]["logits"]  # [10, 128, 512]
        lo = lo.reshape(NO * 128, S)[:VV * VR].T  # [512, 1200]
        outs.append(lo)
    out = np.stack(outs).astype(np.float32)  # [B, S, 1200]
    return out, res


def kernel(**inputs):
    out, _ = kernel_internal(inputs)
    return out
